# revision 48
# baseline (speedup 1.0000x reference)
"""Trainium2 Bass kernel for CRF logZ (nn_CRFModel).

Math: probability-space recurrence with the per-step 1/64 rescale folded
into As = exp(WA - log64) (masked: col BOS = 0, row EOS = 0).

    logZ = ln(a^T prod_t(D_t As^T) p0) + 129*log64,  D_t = diag(exp(emis_t))

The product is evaluated from BOTH ends meeting at t=64: a forward vector
chain p and a backward vector chain gamma run concurrently, fused into a
SINGLE PE matmul per round via the block-diagonal stationary
W = [[As, 0], [0, As^T]] acting on the stacked state X = [p; gamma]
([128, 32]).  63 fused rounds of (PE matmul -> DVE multiply) replace a
naive 128, each round one matmul + one [128,32] multiply.

Emissions: a single fp8(e4m3) copy of E is gathered with
dma_gather(transpose=True) using SIGNED int16 indices based at row 32768
(the Q7 descriptor math is base + stride*signed_idx), so one gather per
word covers the whole 50257-row vocab.  Each pair's fwd+bwd words are
PACKED into one gather (fwd at slots [0,nw), bwd at [nw,2nw)); the GEMM
uses a [128,128] lhsT with ThetaB^T duplicated in both column halves so
one matmul per D-chunk emits fwd tags for fwd words and bwd tags for bwd
words in one pass (the off-blocks are never read).  Gathers are spread
over all 4 SWDGE queues (parallel Q7 descriptor generation); padding
costs nothing: each group is [real words, sentinel idx 0, -1 pads] with
num_idxs_reg = real+1 — the Q7 trims trailing negative idxs, so pads
generate no descriptors while the SBUF layout keeps the padded stride.
The idx table goes through HWDGE (sync/scalar queues) in two pieces so
the first gathers wait only on a tiny head DMA, and the first pair is
small (4 blocks) to cut the pipeline-fill latency before round 0.
"""

import sys

for _p in ("/opt/trn_rl_repo", "/root/.axon_site/_ro/trn_rl_repo"):
    if _p not in sys.path:
        sys.path.insert(0, _p)

import math

import ml_dtypes
import numpy as np

import concourse.bass as bass
import concourse.mybir as mybir
import concourse.tile as tile
from concourse import bacc
from concourse.bass_utils import run_bass_kernel_spmd
from concourse.tile import add_dep_helper

K = 64
V = 50257
D = 512
BT = 256
T = 128
BOS = 62
EOS = 63
N_CORES = 8
B = BT // N_CORES                   # 32 sentences per core
VOFF = 32768                        # signed-idx base row of the E table
LOG64 = math.log(64.0)

# pair p covers NBLK[p] 32-word blocks per direction; fwd t ascending
# from 1+ROUND0[p], bwd t descending from 126-ROUND0[p].  Round r
# (0..62): fwd mult e_{1+r}, bwd mult e_{126-r}.
NBLK = [7, 4, 8, 8, 8, 8, 8, 8, 4]
ROUND0 = [0]
for n in NBLK[:-1]:
    ROUND0.append(ROUND0[-1] + n)
N_ROUNDS = 63
NPAIR = len(NBLK)

F32 = mybir.dt.float32
F16 = mybir.dt.float16
F8 = mybir.dt.float8e4
I16 = mybir.dt.int16


def _pad128(n):
    return -(-n // 128) * 128


NW = [32 * n for n in NBLK]                   # real words per direction
NREAL = [2 * n for n in NW]                   # packed fwd+bwd words
NSLOT = [_pad128(n + 1) for n in NREAL]       # slots incl sentinel+pads
MINI_SLOT = 128                               # 64 real + sentinel + pads
HSLOT = _pad128(NW[0] + 1)                    # pair0 half-gather slots
HEAD_COLS = (MINI_SLOT + 2 * HSLOT) // 16     # idx head: mini + pair0 a/b
REST_COLS = sum(NSLOT[1:]) // 16
# default rotation, except P1 (deadline round 7) swaps with P2 so P1 is
# not queued behind the warmup gather's Q7 descriptor-gen on queue 0
GQUEUE = [1, 2, 3, 1, 0, 2, 3, 0, 1, 2, 3]

_CACHE = {}


def _build():
    nc = bacc.Bacc("TRN2", target_bir_lowering=False, debug=False,
                   num_devices=N_CORES, num_swdge_queues=4)

    idxh_d = nc.dram_tensor("idxh", [128, HEAD_COLS], I16,
                            kind="ExternalInput").ap()
    idxr_d = nc.dram_tensor("idxr", [128, REST_COLS], I16,
                            kind="ExternalInput").ap()
    tht_d = nc.dram_tensor("thT", [128, 512], F16,
                           kind="ExternalInput").ap()
    w_d = nc.dram_tensor("Wbd", [128, 128], F16, kind="ExternalInput").ap()
    ci_d = nc.dram_tensor("cinit", [128, B], F32, kind="ExternalInput").ap()
    e8_d = nc.dram_tensor("E8", [V, D], F8, kind="ExternalInput").ap()
    out_d = nc.dram_tensor("out", [1, B], F32, kind="ExternalOutput").ap()

    with tile.TileContext(nc) as tc:
        with (
            tc.tile_pool(name="const", bufs=1) as cpool,
            tc.tile_pool(name="x", bufs=3) as xpool,
            tc.tile_pool(name="ps_em", bufs=2, space="PSUM") as ps_em,
            tc.tile_pool(name="ps_y", bufs=3, space="PSUM") as ps_y,
            tc.tile_pool(name="ps_z", bufs=1, space="PSUM") as ps_z,
        ):
            # ---- constants (idx head first — gates the first gathers) ----
            idxh = cpool.tile([128, HEAD_COLS], I16, tag="idxh")
            nc.sync.dma_start(idxh[:], idxh_d[:])
            idxr = cpool.tile([128, REST_COLS], I16, tag="idxr")
            nc.scalar.dma_start(idxr[:], idxr_d[:])
            tht = cpool.tile([128, 512], F16, tag="tht")
            nc.sync.dma_start(tht[:], tht_d[:])
            wsb = cpool.tile([128, 128], F16, tag="wsb")
            nc.sync.dma_start(wsb[:], w_d[:])
            cin = cpool.tile([128, B], F32, tag="cin")
            nc.sync.dma_start(cin[:], ci_d[:])
            ones = cpool.tile([128, 1], F32, tag="ones")
            nc.vector.memset(ones[:], 1.0)

            ebase = e8_d[VOFF:VOFF + 2]

            # warmup gather FIRST on the gpsimd queue: it absorbs the
            # one-time Q7 lib-load/first-gather latency, and emitting it
            # before the remaining reg MOVEs lets its long decode overlap
            # them (idx = memset 0 -> row VOFF, harmless scratch reads)
            regs = {127: nc.gpsimd.to_reg(128)}
            widx = cpool.tile([128, 8], I16, tag="widx")
            nc.vector.memset(widx[:], 0)
            wg = cpool.tile([128, 4 * 128], F8, tag="wg")
            nc.gpsimd.dma_gather(
                wg[:].rearrange("p (c w) -> p c w", c=4),
                ebase, widx[:], 128, regs[127], D, transpose=True,
                queue_num=0)

            # one register per distinct real-count (avoids per-gather MOVEs)
            for nreal in {64, NW[0]} | set(NREAL[1:]):
                regs[nreal] = nc.gpsimd.to_reg(nreal + 1)

            # PE p-state warmup: a chain of throwaway matmuls keeps the
            # tensor engine continuously busy from ~7us so it is at full
            # clock when the first real GEMMs arrive (~19us)
            scr = cpool.tile([128, 512], F16, tag="scr")
            nc.vector.memset(scr[:], 0.0)
            ps_w = ps_z.tile([128, 384], F32, tag="warm")
            for _ in range(24):
                nc.tensor.matmul(ps_w[:], lhsT=scr[:, 0:128],
                                 rhs=scr[:, 0:384], start=True, stop=True)

            # ---- all gathers up front; pair0 is split into fwd/bwd half
            # gathers on separate queues (its semaphore gates round 0) ----
            # gt: [mini, P0a, P0b, P1..P8]
            specs = [(MINI_SLOT, 64, 0), (HSLOT, NW[0], MINI_SLOT),
                     (HSLOT, NW[0], MINI_SLOT + HSLOT)]
            off = 0
            for p in range(1, NPAIR):
                specs.append((NSLOT[p], NREAL[p], off))
                off += NSLOT[p]
            gt = [cpool.tile([128, 4 * s[0]], F8, tag=f"g{gi}", name=f"g{gi}")
                  for gi, s in enumerate(specs)]

            def emit_gather(gi, anchor=None):
                nslot, nreal, c0 = specs[gi]
                src = idxh if gi <= 2 else idxr
                ga = nc.gpsimd.dma_gather(
                    gt[gi][:].rearrange("p (c w) -> p c w", c=4),
                    ebase, src[:, c0 // 16:(c0 + nslot) // 16],
                    nslot, regs[nreal], D, transpose=True,
                    queue_num=GQUEUE[gi])
                if anchor is not None:
                    add_dep_helper(ga.ins, anchor.ins,
                                   reason="clear early DMA bandwidth")

            for gi in range(5):           # mini, P0a, P0b, P1, P2
                emit_gather(gi)

            def gemm4(em, gi, ncols, emoff=0, anchors=None):
                """4 accumulating matmuls over packed words [0:ncols);
                the [128,128] lhsT emits fwd tags (0:64) + bwd (64:128)."""
                g = gt[gi]
                v = g[:].rearrange("p (c w j) -> p c w j", c=2, j=2)
                mms = []
                for q in range(4):
                    c16, jj = q // 2, q % 2
                    mm = nc.tensor.matmul(
                        em[:, emoff:emoff + ncols],
                        lhsT=tht[:, 128 * q:128 * (q + 1)],
                        rhs=v[:, c16, 0:ncols, jj],
                        start=(q == 0), stop=(q == 3))
                    mms.append(mm)
                    if anchors is not None:
                        add_dep_helper(mm.ins, anchors[q].ins,
                                       reason="interleave gemm")
                return mms

            last_exp = [None]

            def exp2(em, nw, name):
                """exp both halves: fwd cols [0:nw], bwd cols [nw:2nw];
                order-chained so the scheduler keeps the ACT FIFO in
                pair order (it otherwise defers the bwd half)."""
                ee = cpool.tile([128, nw], F16, tag=name, name=name)
                e1 = nc.scalar.activation(ee[0:64, :], em[0:64, 0:nw],
                                          mybir.ActivationFunctionType.Exp)
                if last_exp[0] is not None:
                    add_dep_helper(e1.ins, last_exp[0].ins,
                                   reason="ACT fifo pair order")
                e2 = nc.scalar.activation(ee[64:128, :], em[64:128, nw:2 * nw],
                                          mybir.ActivationFunctionType.Exp)
                add_dep_helper(e2.ins, e1.ins, reason="ACT fifo pair order")
                last_exp[0] = e2
                return ee

            # ---- mini: fwd t=0 words (slots 0:32), bwd t=127 (32:64) -----
            em0 = ps_em.tile([128, 512], F32, tag="em")
            mini_mms = gemm4(em0, 0, 64)
            ee0 = exp2(em0, B, "ee_mini")
            x = xpool.tile([128, B], F16, tag="x")
            nc.vector.tensor_mul(x[:], cin[:], ee0[:])  # X0 = [p_1; g_127]

            # later pairs' gathers dispatch only once the mini GEMM runs,
            # keeping early DMA-engine bandwidth for the fill-critical ones
            for gi in range(5, len(specs)):
                emit_gather(gi, anchor=mini_mms[0])

            # ---- pair 0 emission (needed from round 0) -------------------
            # bwd half writes the SAME columns on partitions 64:128, so a
            # single exp (not two serial ACTs) feeds round 0
            expe = [None] * NPAIR
            em_p = ps_em.tile([128, 512], F32, tag="em")
            for gi, half in ((1, 0), (2, 64)):
                v = gt[gi][:].rearrange("p (c w j) -> p c w j", c=2, j=2)
                for q in range(4):
                    c16, jj = q // 2, q % 2
                    nc.tensor.matmul(
                        em_p[half:half + 64, 0:NW[0]],
                        lhsT=tht[:, 128 * q:128 * q + 64],
                        rhs=v[:, c16, 0:NW[0], jj],
                        start=(q == 0), stop=(q == 3))
            expe[0] = cpool.tile([128, NW[0]], F16, tag="ee0", name="ee0")
            e0 = nc.scalar.activation(expe[0][:], em_p[:, 0:NW[0]],
                                      mybir.ActivationFunctionType.Exp)
            add_dep_helper(e0.ins, last_exp[0].ins,
                           reason="ACT fifo pair order")
            last_exp[0] = e0

            # ---- 63 fused rounds, next pair's GEMM interleaved -----------
            round_mms = []
            pair = 0
            for r in range(N_ROUNDS):
                if pair + 1 < NPAIR and r == ROUND0[pair + 1]:
                    pair += 1
                k = r - ROUND0[pair]

                y = ps_y.tile([128, B], F32, tag="y")
                mm = nc.tensor.matmul(y[:], lhsT=wsb[:], rhs=x[:],
                                      start=True, stop=True)
                round_mms.append(mm)
                x = xpool.tile([128, B], F16, tag="x")
                nc.vector.tensor_mul(x[:], y[:],
                                     expe[pair][:, B * k:B * (k + 1)])

                # pair p+1's emission, anchored spread across this pair's
                # rounds (keeps PE gap-free -> MID p-state)
                if k == NBLK[pair] - 2 and pair + 1 < NPAIR:
                    p = pair + 1
                    em_n = ps_em.tile([128, 512], F32, tag="em",
                                      name=f"em{p}")
                    r0 = ROUND0[pair] + (2 if pair == 0 else 0)
                    anch = [round_mms[min(r0 + (q * (k + 1)) // 4, r)]
                            for q in range(4)]
                    gemm4(em_n, 2 + p, NREAL[p], anchors=anch)
                    expe[p] = exp2(em_n, NW[p], f"ee{p}")

            # ---- finale: Z~ = gamma64^T As^T p64 -------------------------
            yf = ps_y.tile([128, B], F32, tag="y")
            nc.tensor.matmul(yf[64:128, :], lhsT=wsb[0:64, 0:64],
                             rhs=x[0:64, :], start=True, stop=True)
            z1 = cpool.tile([128, B], F32, tag="z1")
            nc.vector.tensor_mul(z1[64:128, :], yf[64:128, :], x[64:128, :])
            z2 = ps_z.tile([1, B], F32, tag="z")
            nc.tensor.matmul(z2[:], lhsT=ones[64:128, :], rhs=z1[64:128, :],
                             start=True, stop=True)
            res = cpool.tile([1, B], F32, tag="res")
            nc.vector.tensor_scalar_add(res[:], z2[:], 0.0)
            nc.sync.dma_start(out_d[:], res[:])

    nc.compile()
    return nc


def _get_nc():
    if "nc" not in _CACHE:
        _CACHE["nc"] = _build()
    return _CACHE["nc"]


def _wrap16(vals):
    """slot j -> partition j%16, col j//16; replicated to all 8 Q7 cores."""
    a = np.asarray(vals, np.int16).reshape(-1, 16).T
    return np.tile(a, (8, 1))


def _make_in_maps(words, WA, ThetaB, E):
    words = np.asarray(words)
    WA = np.asarray(WA, np.float32)
    ThetaB = np.asarray(ThetaB, np.float32)
    E = np.asarray(E, np.float32)

    As = np.exp(WA - LOG64)
    As[:, BOS] = 0.0
    As[EOS, :] = 0.0
    W = np.zeros((128, 128), np.float16)
    W[:64, :64] = As
    W[64:, 64:] = As.T
    cin = np.empty((128, B), np.float32)
    cin[:64, :] = As[BOS, :][:, None]      # p_1 = e_0 * As[BOS, :]
    cin[64:, :] = As[:, EOS][:, None]      # gamma_127 = e_127 * As[:, EOS]

    # ThetaB^T in the gather's 16-bit-interleaved layout, duplicated into
    # both lhsT column halves: chunk q=(2*c16+j):
    #   tht[p, 128q + m] = ThetaB[m % 64, 256*c16 + 2p + j]
    tht = np.empty((128, 512), np.float16)
    p_ar = np.arange(128)
    for q in range(4):
        c16, j = q // 2, q % 2
        blk = ThetaB[:, 256 * c16 + 2 * p_ar + j].T          # [128, 64]
        tht[:, 128 * q:128 * q + 64] = blk
        tht[:, 128 * q + 64:128 * (q + 1)] = blk
    E8 = np.ascontiguousarray(E.astype(ml_dtypes.float8_e4m3fn))

    in_maps = []
    for c in range(N_CORES):
        wb = words[c * B:(c + 1) * B].astype(np.int64)        # [32, 128]

        def block(f_ts, b_ts, pad_to):
            wf = wb[:, f_ts].T.reshape(-1)
            wbk = wb[:, b_ts].T.reshape(-1)
            iv = (np.concatenate([wf, wbk]) - VOFF).astype(np.int16)
            out = np.full(pad_to, -1, np.int16)   # trailing pads trimmed
            out[:len(iv)] = iv
            out[len(iv)] = 0                      # sentinel keeps reg count
            return out

        parts = [block([0], [127], MINI_SLOT)]
        f0 = list(range(1, 1 + NBLK[0]))
        b0 = list(range(126, 126 - NBLK[0], -1))
        parts.append(block(f0, [], HSLOT))        # pair0 fwd half
        parts.append(block([], b0, HSLOT))        # pair0 bwd half
        t = 1 + NBLK[0]
        for p in range(1, NPAIR):
            f_ts = list(range(t, t + NBLK[p]))
            b_ts = list(range(127 - t, 127 - t - NBLK[p], -1))
            parts.append(block(f_ts, b_ts, NSLOT[p]))
            t += NBLK[p]
        idxh = np.hstack([_wrap16(b) for b in parts[:3]])
        idxr = np.hstack([_wrap16(b) for b in parts[3:]])
        in_maps.append({
            "idxh": np.ascontiguousarray(idxh),
            "idxr": np.ascontiguousarray(idxr),
            "thT": tht, "Wbd": W, "cinit": cin, "E8": E8,
        })
    return in_maps


def kernel(words, WA, ThetaB, E):
    nc = _get_nc()
    in_maps = _make_in_maps(words, WA, ThetaB, E)
    res = run_bass_kernel_spmd(nc, in_maps, list(range(N_CORES)))
    z = np.concatenate([res.results[c]["out"][0] for c in range(N_CORES)])
    return (np.log(z.astype(np.float64)) + 129 * LOG64).astype(np.float32)


# revision 49
# speedup vs baseline: 1.1612x; 1.1612x over previous
"""Trainium2 Bass kernel for CRF logZ (nn_CRFModel).

Math: probability-space recurrence with the per-step 1/64 rescale folded
into As = exp(WA - log64) (masked: col BOS = 0, row EOS = 0).

    logZ = ln(a^T prod_t(D_t As^T) p0) + 129*log64,  D_t = diag(exp(emis_t))

The product is evaluated from BOTH ends meeting at t=64: a forward vector
chain p and a backward vector chain gamma run concurrently, fused into a
SINGLE PE matmul per round via the block-diagonal stationary
W = [[As, 0], [0, As^T]] acting on the stacked state X = [p; gamma]
([128, 32]).  63 fused rounds of (PE matmul -> DVE multiply) replace a
naive 128, each round one matmul + one [128,32] multiply.

Emissions: a single fp8(e4m3) copy of E is gathered with
dma_gather(transpose=True) using SIGNED int16 indices based at row 32768
(the Q7 descriptor math is base + stride*signed_idx), so one gather per
word covers the whole 50257-row vocab.  Each pair's fwd+bwd words are
PACKED into one gather (fwd at slots [0,nw), bwd at [nw,2nw)); the GEMM
uses a [128,128] lhsT with ThetaB^T duplicated in both column halves so
one matmul per D-chunk emits fwd tags for fwd words and bwd tags for bwd
words in one pass (the off-blocks are never read).  Gathers are spread
over all 4 SWDGE queues (parallel Q7 descriptor generation); padding
costs nothing: each group is [real words, sentinel idx 0, -1 pads] with
num_idxs_reg = real+1 — the Q7 trims trailing negative idxs, so pads
generate no descriptors while the SBUF layout keeps the padded stride.
The idx table goes through HWDGE (sync/scalar queues) in two pieces so
the first gathers wait only on a tiny head DMA, and the first pair is
small (4 blocks) to cut the pipeline-fill latency before round 0.
"""

import sys

for _p in ("/opt/trn_rl_repo", "/root/.axon_site/_ro/trn_rl_repo"):
    if _p not in sys.path:
        sys.path.insert(0, _p)

import math

import ml_dtypes
import numpy as np

import concourse.bass as bass
import concourse.mybir as mybir
import concourse.tile as tile
from concourse import bacc
from concourse.bass_utils import run_bass_kernel_spmd
from concourse.tile import add_dep_helper

K = 64
V = 50257
D = 512
BT = 256
T = 128
BOS = 62
EOS = 63
N_CORES = 8
B = BT // N_CORES                   # 32 sentences per core
VOFF = 32768                        # signed-idx base row of the E table
LOG64 = math.log(64.0)

# pair p covers NBLK[p] 32-word blocks per direction; fwd t ascending
# from 1+ROUND0[p], bwd t descending from 126-ROUND0[p].  Round r
# (0..62): fwd mult e_{1+r}, bwd mult e_{126-r}.
NBLK = [7, 4, 8, 8, 8, 8, 8, 8, 4]
ROUND0 = [0]
for n in NBLK[:-1]:
    ROUND0.append(ROUND0[-1] + n)
N_ROUNDS = 63
NPAIR = len(NBLK)

F32 = mybir.dt.float32
F16 = mybir.dt.float16
F8 = mybir.dt.float8e4
I16 = mybir.dt.int16


def _pad128(n):
    return -(-n // 128) * 128


NW = [32 * n for n in NBLK]                   # real words per direction
NREAL = [2 * n for n in NW]                   # packed fwd+bwd words
NSLOT = [_pad128(n + 1) for n in NREAL]       # slots incl sentinel+pads
MINI_SLOT = 128                               # 64 real + sentinel + pads
HSLOT = _pad128(NW[0] + 1)                    # pair0 half-gather slots
HEAD_COLS = (MINI_SLOT + 2 * HSLOT) // 16     # idx head: mini + pair0 a/b
REST_COLS = sum(NSLOT[1:]) // 16
# default rotation, except P1 (deadline round 7) swaps with P2 so P1 is
# not queued behind the warmup gather's Q7 descriptor-gen on queue 0
GQUEUE = [1, 2, 3, 1, 0, 2, 3, 0, 1, 2, 3]

_CACHE = {}


def _build():
    nc = bacc.Bacc("TRN2", target_bir_lowering=False, debug=False,
                   num_devices=N_CORES, num_swdge_queues=4)

    idxh_d = nc.dram_tensor("idxh", [128, HEAD_COLS], I16,
                            kind="ExternalInput").ap()
    idxr_d = nc.dram_tensor("idxr", [128, REST_COLS], I16,
                            kind="ExternalInput").ap()
    tht_d = nc.dram_tensor("thT", [128, 512], F16,
                           kind="ExternalInput").ap()
    w_d = nc.dram_tensor("Wbd", [128, 128], F16, kind="ExternalInput").ap()
    ci_d = nc.dram_tensor("cinit", [128, B], F32, kind="ExternalInput").ap()
    e8_d = nc.dram_tensor("E8", [V, D], F8, kind="ExternalInput").ap()
    out_d = nc.dram_tensor("out", [1, B], F32, kind="ExternalOutput").ap()

    with tile.TileContext(nc) as tc:
        with (
            tc.tile_pool(name="const", bufs=1) as cpool,
            tc.tile_pool(name="x", bufs=3) as xpool,
            tc.tile_pool(name="ps_em", bufs=2, space="PSUM") as ps_em,
            tc.tile_pool(name="ps_y", bufs=3, space="PSUM") as ps_y,
            tc.tile_pool(name="ps_z", bufs=1, space="PSUM") as ps_z,
        ):
            # ---- constants (idx head first — gates the first gathers) ----
            idxh = cpool.tile([128, HEAD_COLS], I16, tag="idxh")
            nc.sync.dma_start(idxh[:], idxh_d[:])
            idxr = cpool.tile([128, REST_COLS], I16, tag="idxr")
            nc.scalar.dma_start(idxr[:], idxr_d[:])
            tht = cpool.tile([128, 512], F16, tag="tht")
            nc.sync.dma_start(tht[:], tht_d[:])
            wsb = cpool.tile([128, 128], F16, tag="wsb")
            nc.sync.dma_start(wsb[:], w_d[:])
            cin = cpool.tile([128, B], F32, tag="cin")
            nc.sync.dma_start(cin[:], ci_d[:])
            ones = cpool.tile([128, 1], F32, tag="ones")
            nc.vector.memset(ones[:], 1.0)

            ebase = e8_d[VOFF:VOFF + 2]

            # one register per distinct real-count (avoids per-gather MOVEs)
            regs = {}
            for nreal in {64, 127, NW[0]} | set(NREAL[1:]):
                regs[nreal] = nc.gpsimd.to_reg(nreal + 1)

            # warmup gather: absorbs the one-time Q7 lib-load/first-gather
            # latency before the real idx data even arrives (idx = memset 0
            # -> row VOFF, harmless reads into a scratch tile)
            widx = cpool.tile([128, 8], I16, tag="widx")
            nc.vector.memset(widx[:], 0)
            wg = cpool.tile([128, 4 * 128], F8, tag="wg")
            nc.gpsimd.dma_gather(
                wg[:].rearrange("p (c w) -> p c w", c=4),
                ebase, widx[:], 128, regs[127], D, transpose=True,
                queue_num=0)

            # PE p-state warmup: a chain of throwaway matmuls keeps the
            # tensor engine continuously busy from ~7us so it is at full
            # clock when the first real GEMMs arrive (~19us)
            scr = cpool.tile([128, 512], F16, tag="scr")
            nc.vector.memset(scr[:], 0.0)
            ps_w = ps_z.tile([128, 384], F32, tag="warm")
            for _ in range(24):
                nc.tensor.matmul(ps_w[:], lhsT=scr[:, 0:128],
                                 rhs=scr[:, 0:384], start=True, stop=True)

            # ---- all gathers up front; pair0 is split into fwd/bwd half
            # gathers on separate queues (its semaphore gates round 0) ----
            # gt: [mini, P0a, P0b, P1..P8]
            specs = [(MINI_SLOT, 64, 0), (HSLOT, NW[0], MINI_SLOT),
                     (HSLOT, NW[0], MINI_SLOT + HSLOT)]
            off = 0
            for p in range(1, NPAIR):
                specs.append((NSLOT[p], NREAL[p], off))
                off += NSLOT[p]
            gt = [cpool.tile([128, 4 * s[0]], F8, tag=f"g{gi}", name=f"g{gi}")
                  for gi, s in enumerate(specs)]

            def emit_gather(gi, anchor=None):
                nslot, nreal, c0 = specs[gi]
                src = idxh if gi <= 2 else idxr
                ga = nc.gpsimd.dma_gather(
                    gt[gi][:].rearrange("p (c w) -> p c w", c=4),
                    ebase, src[:, c0 // 16:(c0 + nslot) // 16],
                    nslot, regs[nreal], D, transpose=True,
                    queue_num=GQUEUE[gi])
                if anchor is not None:
                    add_dep_helper(ga.ins, anchor.ins,
                                   reason="clear early DMA bandwidth")

            for gi in range(5):           # mini, P0a, P0b, P1, P2
                emit_gather(gi)

            def gemm4(em, gi, ncols, emoff=0, anchors=None):
                """4 accumulating matmuls over packed words [0:ncols);
                the [128,128] lhsT emits fwd tags (0:64) + bwd (64:128)."""
                g = gt[gi]
                v = g[:].rearrange("p (c w j) -> p c w j", c=2, j=2)
                mms = []
                for q in range(4):
                    c16, jj = q // 2, q % 2
                    mm = nc.tensor.matmul(
                        em[:, emoff:emoff + ncols],
                        lhsT=tht[:, 128 * q:128 * (q + 1)],
                        rhs=v[:, c16, 0:ncols, jj],
                        start=(q == 0), stop=(q == 3))
                    mms.append(mm)
                    if anchors is not None:
                        add_dep_helper(mm.ins, anchors[q].ins,
                                       reason="interleave gemm")
                return mms

            last_exp = [None]

            def exp2(em, nw, name):
                """exp both halves: fwd cols [0:nw], bwd cols [nw:2nw];
                order-chained so the scheduler keeps the ACT FIFO in
                pair order (it otherwise defers the bwd half)."""
                ee = cpool.tile([128, nw], F16, tag=name, name=name)
                e1 = nc.scalar.activation(ee[0:64, :], em[0:64, 0:nw],
                                          mybir.ActivationFunctionType.Exp)
                if last_exp[0] is not None:
                    add_dep_helper(e1.ins, last_exp[0].ins,
                                   reason="ACT fifo pair order")
                e2 = nc.scalar.activation(ee[64:128, :], em[64:128, nw:2 * nw],
                                          mybir.ActivationFunctionType.Exp)
                add_dep_helper(e2.ins, e1.ins, reason="ACT fifo pair order")
                last_exp[0] = e2
                return ee

            # ---- mini: fwd t=0 words (slots 0:32), bwd t=127 (32:64) -----
            em0 = ps_em.tile([128, 512], F32, tag="em")
            mini_mms = gemm4(em0, 0, 64)
            ee0 = exp2(em0, B, "ee_mini")
            x = xpool.tile([128, B], F16, tag="x")
            nc.vector.tensor_mul(x[:], cin[:], ee0[:])  # X0 = [p_1; g_127]

            # later pairs' gathers dispatch only once the mini GEMM runs,
            # keeping early DMA-engine bandwidth for the fill-critical ones
            for gi in range(5, len(specs)):
                emit_gather(gi, anchor=mini_mms[0])

            # ---- pair 0 emission (needed from round 0) -------------------
            # bwd half writes the SAME columns on partitions 64:128, so a
            # single exp (not two serial ACTs) feeds round 0
            expe = [None] * NPAIR
            em_p = ps_em.tile([128, 512], F32, tag="em")
            for gi, half in ((1, 0), (2, 64)):
                v = gt[gi][:].rearrange("p (c w j) -> p c w j", c=2, j=2)
                for q in range(4):
                    c16, jj = q // 2, q % 2
                    nc.tensor.matmul(
                        em_p[half:half + 64, 0:NW[0]],
                        lhsT=tht[:, 128 * q:128 * q + 64],
                        rhs=v[:, c16, 0:NW[0], jj],
                        start=(q == 0), stop=(q == 3))
            expe[0] = cpool.tile([128, NW[0]], F16, tag="ee0", name="ee0")
            e0 = nc.scalar.activation(expe[0][:], em_p[:, 0:NW[0]],
                                      mybir.ActivationFunctionType.Exp)
            add_dep_helper(e0.ins, last_exp[0].ins,
                           reason="ACT fifo pair order")
            last_exp[0] = e0

            # ---- 63 fused rounds, next pair's GEMM interleaved -----------
            round_mms = []
            pair = 0
            for r in range(N_ROUNDS):
                if pair + 1 < NPAIR and r == ROUND0[pair + 1]:
                    pair += 1
                k = r - ROUND0[pair]

                y = ps_y.tile([128, B], F32, tag="y")
                mm = nc.tensor.matmul(y[:], lhsT=wsb[:], rhs=x[:],
                                      start=True, stop=True)
                round_mms.append(mm)
                x = xpool.tile([128, B], F16, tag="x")
                nc.vector.tensor_mul(x[:], y[:],
                                     expe[pair][:, B * k:B * (k + 1)])

                # pair p+1's emission, anchored spread across this pair's
                # rounds (keeps PE gap-free -> MID p-state)
                if k == NBLK[pair] - 2 and pair + 1 < NPAIR:
                    p = pair + 1
                    em_n = ps_em.tile([128, 512], F32, tag="em",
                                      name=f"em{p}")
                    r0 = ROUND0[pair] + (2 if pair == 0 else 0)
                    anch = [round_mms[min(r0 + (q * (k + 1)) // 4, r)]
                            for q in range(4)]
                    gemm4(em_n, 2 + p, NREAL[p], anchors=anch)
                    expe[p] = exp2(em_n, NW[p], f"ee{p}")

            # ---- finale: Z~ = gamma64^T As^T p64 -------------------------
            yf = ps_y.tile([128, B], F32, tag="y")
            nc.tensor.matmul(yf[64:128, :], lhsT=wsb[0:64, 0:64],
                             rhs=x[0:64, :], start=True, stop=True)
            z1 = cpool.tile([128, B], F32, tag="z1")
            nc.vector.tensor_mul(z1[64:128, :], yf[64:128, :], x[64:128, :])
            z2 = ps_z.tile([1, B], F32, tag="z")
            nc.tensor.matmul(z2[:], lhsT=ones[64:128, :], rhs=z1[64:128, :],
                             start=True, stop=True)
            res = cpool.tile([1, B], F32, tag="res")
            nc.vector.tensor_scalar_add(res[:], z2[:], 0.0)
            nc.sync.dma_start(out_d[:], res[:])

    nc.compile()
    return nc


def _get_nc():
    if "nc" not in _CACHE:
        _CACHE["nc"] = _build()
    return _CACHE["nc"]


def _wrap16(vals):
    """slot j -> partition j%16, col j//16; replicated to all 8 Q7 cores."""
    a = np.asarray(vals, np.int16).reshape(-1, 16).T
    return np.tile(a, (8, 1))


def _make_in_maps(words, WA, ThetaB, E):
    words = np.asarray(words)
    WA = np.asarray(WA, np.float32)
    ThetaB = np.asarray(ThetaB, np.float32)
    E = np.asarray(E, np.float32)

    As = np.exp(WA - LOG64)
    As[:, BOS] = 0.0
    As[EOS, :] = 0.0
    W = np.zeros((128, 128), np.float16)
    W[:64, :64] = As
    W[64:, 64:] = As.T
    cin = np.empty((128, B), np.float32)
    cin[:64, :] = As[BOS, :][:, None]      # p_1 = e_0 * As[BOS, :]
    cin[64:, :] = As[:, EOS][:, None]      # gamma_127 = e_127 * As[:, EOS]

    # ThetaB^T in the gather's 16-bit-interleaved layout, duplicated into
    # both lhsT column halves: chunk q=(2*c16+j):
    #   tht[p, 128q + m] = ThetaB[m % 64, 256*c16 + 2p + j]
    tht = np.empty((128, 512), np.float16)
    p_ar = np.arange(128)
    for q in range(4):
        c16, j = q // 2, q % 2
        blk = ThetaB[:, 256 * c16 + 2 * p_ar + j].T          # [128, 64]
        tht[:, 128 * q:128 * q + 64] = blk
        tht[:, 128 * q + 64:128 * (q + 1)] = blk
    E8 = np.ascontiguousarray(E.astype(ml_dtypes.float8_e4m3fn))

    in_maps = []
    for c in range(N_CORES):
        wb = words[c * B:(c + 1) * B].astype(np.int64)        # [32, 128]

        def block(f_ts, b_ts, pad_to):
            wf = wb[:, f_ts].T.reshape(-1)
            wbk = wb[:, b_ts].T.reshape(-1)
            iv = (np.concatenate([wf, wbk]) - VOFF).astype(np.int16)
            out = np.full(pad_to, -1, np.int16)   # trailing pads trimmed
            out[:len(iv)] = iv
            out[len(iv)] = 0                      # sentinel keeps reg count
            return out

        parts = [block([0], [127], MINI_SLOT)]
        f0 = list(range(1, 1 + NBLK[0]))
        b0 = list(range(126, 126 - NBLK[0], -1))
        parts.append(block(f0, [], HSLOT))        # pair0 fwd half
        parts.append(block([], b0, HSLOT))        # pair0 bwd half
        t = 1 + NBLK[0]
        for p in range(1, NPAIR):
            f_ts = list(range(t, t + NBLK[p]))
            b_ts = list(range(127 - t, 127 - t - NBLK[p], -1))
            parts.append(block(f_ts, b_ts, NSLOT[p]))
            t += NBLK[p]
        idxh = np.hstack([_wrap16(b) for b in parts[:3]])
        idxr = np.hstack([_wrap16(b) for b in parts[3:]])
        in_maps.append({
            "idxh": np.ascontiguousarray(idxh),
            "idxr": np.ascontiguousarray(idxr),
            "thT": tht, "Wbd": W, "cinit": cin, "E8": E8,
        })
    return in_maps


def kernel(words, WA, ThetaB, E):
    nc = _get_nc()
    in_maps = _make_in_maps(words, WA, ThetaB, E)
    res = run_bass_kernel_spmd(nc, in_maps, list(range(N_CORES)))
    z = np.concatenate([res.results[c]["out"][0] for c in range(N_CORES)])
    return (np.log(z.astype(np.float64)) + 129 * LOG64).astype(np.float32)
